# revision 1
# baseline (speedup 1.0000x reference)
"""Trainium2 Bass kernel for the 2-qubit quantum-circuit batch evaluation.

Reference semantics (per batch row, x = [x0, x1], scalar theta):
    state = RY(theta) @ CNOT @ (RY(x0)|0> ⊗ RY(x1)|0>)
    out = (<Z>, +1)/2 for each qubit.

Algebraically this reduces (product/half-angle identities) to:
    out0 = 0.5 + 0.5*cos(theta)*cos(x0) - 0.5*sin(theta)*sin(x0)*sin(x1)
    out1 = 0.5 + 0.5*cos(x0)*cos(x1)

So the device kernel is a pure streaming map: per element-pair it needs
sin/cos of both angles (ScalarE `Sin` activation; cos via bias=pi/2) and a
handful of elementwise combines (VectorE / ScalarE), making it HBM-bound.

Sharding: pure data parallel over 8 NeuronCores; theta-derived scalars
(0.5*cos(theta), -0.5*sin(theta)) are computed on host and passed as a tiny
replicated [128, 2] constant tensor.
"""

import numpy as np

import concourse.bass as bass
import concourse.mybir as mybir
from concourse.alu_op_type import AluOpType
from concourse.bacc import Bacc
from concourse.tile import TileContext
from concourse import bass_utils

N_CORES = 8
B = 8388608
BC = B // N_CORES            # rows per core
ELEMS = BC * 2               # flat f32 elements per core
P = 128                      # SBUF partitions
F = 4096                     # free elems per partition per tile
T = ELEMS // (P * F)         # tiles per core
HALF_PI = float(np.pi / 2)
MAGIC = float(1.5 * 2**23)   # f32 round-to-nearest-int magic constant

_CACHE = {}


def _build_nc():
    # Bacc (not raw Bass): its compile() pass splits multi-wait sync_info into
    # EventSemaphore instructions — TRN2 allows at most 1 wait per instruction.
    nc = Bacc()
    x = nc.dram_tensor("x", [BC, 2], mybir.dt.float32, kind="ExternalInput")
    consts = nc.dram_tensor("consts", [P, 5], mybir.dt.float32, kind="ExternalInput")
    out = nc.dram_tensor("out", [BC, 2], mybir.dt.float32, kind="ExternalOutput")

    x_t = x[:].flatten().rearrange("(n p f) -> n p f", p=P, f=F)
    o_t = out[:].flatten().rearrange("(n p f) -> n p f", p=P, f=F)

    f32 = mybir.dt.float32
    Sin = mybir.ActivationFunctionType.Sin
    Ident = mybir.ActivationFunctionType.Identity

    with TileContext(nc) as tc:
        with tc.tile_pool(name="cpool", bufs=1) as cpool, \
             tc.tile_pool(name="io", bufs=2) as io, \
             tc.tile_pool(name="work", bufs=2) as work:
            ct = cpool.tile([P, 5], f32)
            nc.sync.dma_start(out=ct[:], in_=consts[:])
            hc = ct[:, 0:1]      # 0.5*cos(theta)
            ns = ct[:, 1:2]      # -0.5*sin(theta)
            half = ct[:, 2:3]    # 0.5
            halfpi = ct[:, 3:4]  # pi/2
            negpi = ct[:, 4:5]   # -pi

            for i in range(T):
                xt = io.tile([P, F], f32, tag="xt")
                nc.sync.dma_start(out=xt[:], in_=x_t[i])

                # Range reduction: ACT Sin is only accurate for |arg| <= pi,
                # but x spans ~±17. Magic-number rounding (mod isn't valid DVE
                # ISA): t = x/(2pi) + 1.5*2^23 forces round-to-nearest-int in
                # the mantissa; k2 = (t - MAGIC)*(-2pi) = -2pi*round(x/2pi);
                # y = x + k2 in [-pi, pi]. sin(x) = Sin(y); cos by evenness:
                # cos(x) = Sin(pi/2 - |y|), abs split across ACT/DVE to balance.
                t = work.tile([P, F], f32, tag="t")
                y = work.tile([P, F], f32, tag="y")
                nc.vector.tensor_scalar(
                    t[:], xt[:], float(1.0 / (2 * np.pi)), MAGIC,
                    AluOpType.mult, AluOpType.add,
                )
                # k2 in place of t, then y = x + k2
                nc.vector.tensor_scalar(
                    t[:], t[:], MAGIC, float(-2 * np.pi),
                    AluOpType.subtract, AluOpType.mult,
                )
                nc.vector.tensor_tensor(y[:], xt[:], t[:], AluOpType.add)
                # S reuses t's slots (t is dead after y)
                S = work.tile([P, F], f32, tag="t")
                nc.scalar.activation(S[:], y[:], Sin)
                # |y| then C = Sin(pi/2 - |y|), both in place of y
                nc.scalar.activation(y[:], y[:], mybir.ActivationFunctionType.Abs)
                C = y
                nc.scalar.activation(C[:], y[:], Sin, bias=halfpi, scale=-1.0)

                Sv = S[:].rearrange("p (k two) -> p k two", two=2)
                Cv = C[:].rearrange("p (k two) -> p k two", two=2)
                o = io.tile([P, F], f32, tag="o")
                ov = o[:].rearrange("p (k two) -> p k two", two=2)

                m = work.tile([P, F // 2], f32, tag="m")
                g = work.tile([P, F // 2], f32, tag="g")
                a = work.tile([P, F // 2], f32, tag="a")
                m2 = m  # m2 = Copy(m*ns) in place

                # m = sin(x0)*sin(x1); g = cos(x0)*cos(x1)
                nc.vector.tensor_tensor(m[:], Sv[:, :, 0], Sv[:, :, 1], AluOpType.mult)
                nc.vector.tensor_tensor(g[:], Cv[:, :, 0], Cv[:, :, 1], AluOpType.mult)
                # a = 0.5*cos(theta)*cos(x0) + 0.5   (ScalarE, runtime scale)
                nc.scalar.activation(a[:], Cv[:, :, 0], Ident, bias=half, scale=hc)
                # m2 = -0.5*sin(theta)*m (ACT Copy, runtime scale);
                # out1 = 0.5*g + 0.5 (ACT); out0 = a + m2 (DVE).
                Copy = mybir.ActivationFunctionType.Copy
                nc.scalar.activation(m2[:], m[:], Copy, scale=ns)
                nc.scalar.activation(ov[:, :, 1], g[:], Ident, bias=half, scale=half)
                nc.vector.tensor_tensor(ov[:, :, 0], a[:], m2[:], AluOpType.add)

                nc.sync.dma_start(out=o_t[i], in_=o[:])
    nc.compile()
    return nc


def _run(in_maps, trace=False, trace_cores=None):
    if "nc" not in _CACHE:
        _CACHE["nc"] = _build_nc()
    return bass_utils.run_bass_kernel_spmd(
        _CACHE["nc"],
        in_maps,
        core_ids=list(range(N_CORES)),
        trace=trace,
        trace_cores=trace_cores,
    )


def kernel(x, theta, _trace=False, _trace_cores=None):
    x = np.ascontiguousarray(np.asarray(x, dtype=np.float32))
    theta = np.asarray(theta, dtype=np.float32)
    assert x.shape == (B, 2), x.shape

    th = float(theta.reshape(-1)[0])
    consts = np.empty((P, 5), dtype=np.float32)
    consts[:, 0] = 0.5 * np.cos(th)
    consts[:, 1] = -0.5 * np.sin(th)
    consts[:, 2] = 0.5
    consts[:, 3] = HALF_PI
    consts[:, 4] = -np.pi

    shards = x.reshape(N_CORES, BC, 2)
    in_maps = [{"x": shards[c], "consts": consts} for c in range(N_CORES)]

    res = _run(in_maps, trace=_trace, trace_cores=_trace_cores)
    _CACHE["last_results"] = res
    out = np.concatenate([res.results[c]["out"] for c in range(N_CORES)], axis=0)
    return out



# revision 2
# speedup vs baseline: 1.0140x; 1.0140x over previous
"""Trainium2 Bass kernel for the 2-qubit quantum-circuit batch evaluation.

Math (per row, x = [x0, x1], scalar theta):
    out0 = 0.5 + 0.5*cos(theta)*cos(x0) - 0.5*sin(theta)*sin(x0)*sin(x1)
    out1 = 0.5 + 0.5*cos(x0)*cos(x1)

Design (all hw-measured; see transcript for the measurement history):
  * theta scalars baked as immediates (compile per theta; harness calls once).
  * range reduction with ZERO dedicated DVE rounding ops:
      - ACT Identity pass computes t16 = fp16(x/(2pi) + 1536); the fp16
        output-convert performs round-to-nearest-int (fp16 magic).
      - one fused cody_waite_cascade with MIXED dtypes (fp32 x, fp16 t16)
        gives y' = x - t16*2pi ~ y - 3072pi (exact-enough: t16*c1 is exact
        in fp32 since t16*402 < 2^24).
      - the -3072pi shift is cancelled FOR FREE by the Sin bias:
        sin(x) = Sin(y' + B), B = fp32(1536*(c1+c2+c3)); total err ~1e-3
        vs 2e-2 tolerance. Boundary k-flips are harmless (Sin accurate to
        ~1.25pi, hw-verified).
  * cos via half-angle square: cos(x) = 1 - 2*sin^2(y/2); the half-angle
    Sin pass is free (ACT scale=0.5, bias=B/2).
  * ACT Sin passes write fp16 PLANES (deinterleave x0/x1 via strided read,
    +7%) so all DVE combines run in 16-bit 2x/4x modes.
  * 3-stage software pipeline (engines execute in program order):
      a(i):   DMA in, ACT t16
      b(i-1): DVE cody, ACT Sin -> S planes, ACT Sin/2 -> Ha planes
      c(i-2): DVE squares+combines+out0, ACT out1, DMA out
    This keeps each engine's in-order stream free of same-tile round trips.

Per [128,2048] tile (8/core): DVE ~7.4us, ACT ~7.3us, DMA 2MB ~5.9us.
"""

import os
import numpy as np

import concourse.bass as bass
import concourse.mybir as mybir
from concourse.alu_op_type import AluOpType
from concourse.bacc import Bacc
from concourse.tile import TileContext
from concourse import bass_utils

N_CORES = 8
B = 8388608
BC = B // N_CORES            # rows per core
ELEMS = BC * 2               # flat f32 elements per core
P = 128
F = int(os.environ.get("TILE_F", "2048"))
H = F // 2
T = ELEMS // (P * F)         # tiles per core

INV_2PI = float(1.0 / (2 * np.pi))
M16 = 1536.0                 # fp16 round-to-int magic (1.5*2^10)
CW1 = float(np.float32(6.28125))
CW2 = float(np.float32(2 * np.pi - 6.28125))
CW3 = float(2 * np.pi - 6.28125 - float(np.float32(2 * np.pi - 6.28125)))
_CSUM = np.float64(CW1) + np.float64(CW2) + np.float64(CW3)
BIAS = float(np.float32(1536.0 * _CSUM))        # cancels the -3072pi shift
BIAS2 = float(np.float32(768.0 * _CSUM))

f32 = mybir.dt.float32
f16 = mybir.dt.float16
Sin = mybir.ActivationFunctionType.Sin
Sq = mybir.ActivationFunctionType.Square
Ident = mybir.ActivationFunctionType.Identity

_CACHE = {}


def _reg_consts(nc, vals):
    for v in vals:
        v = float(v)
        if (f32, v) in nc.const_aps.aps:
            continue
        t = nc.alloc_sbuf_tensor(f"const-float32-{v}", [128, 1], f32)
        nc.vector.memset(t.ap(), v)
        nc.const_aps.aps[(f32, v)] = t.ap()
    nc.all_engine_barrier()


def _build_nc(hc, ns):
    """hc = 0.5*cos(theta), ns = -0.5*sin(theta) baked as immediates."""
    nc = Bacc()
    _reg_consts(nc, [M16, BIAS, BIAS2, 0.5])
    x = nc.dram_tensor("x", [BC, 2], f32, kind="ExternalInput")
    out = nc.dram_tensor("out", [BC, 2], f32, kind="ExternalOutput")

    x_t = x[:].flatten().rearrange("(n p f) -> n p f", p=P, f=F)
    o_t = out[:].flatten().rearrange("(n p f) -> n p f", p=P, f=F)

    with TileContext(nc) as tc:
        with tc.tile_pool(name="io", bufs=8) as io, \
             tc.tile_pool(name="rk", bufs=6) as rk, \
             tc.tile_pool(name="pb", bufs=4) as pb:
            xts, t16s, ys, stash = {}, {}, {}, {}

            # dummy ACT op: trigger the Sin table load during the first DMA
            warm = rk.tile([P, 16], f16, tag="warm")
            nc.vector.memset(warm[:], 0.0)
            nc.scalar.activation(warm[:], warm[:], Sin)

            def stage_a(i):
                xt = io.tile([P, F], f32, tag="x")
                nc.sync.dma_start(out=xt[:], in_=x_t[i])
                # ACT magic-round: t16 = fp16(x/(2pi) + 1536)
                t16 = pb.tile([P, F], f16, tag="t")
                nc.scalar.activation(t16[:], xt[:], Ident,
                                     bias=M16, scale=INV_2PI)
                xts[i] = xt
                t16s[i] = t16

            def stage_b(j):
                xt = xts.pop(j)
                t16 = t16s.pop(j)
                # y' = x - t16*2pi ~ y - 3072pi  (mixed-dtype cody)
                y = rk.tile([P, F], f32, tag="y")
                nc.vector.cody_waite_cascade(y[:], xt[:], t16[:],
                                             CW1, CW2, CW3)
                # fp16 planes: S = [sin(x0) | sin(x1)] via Sin(y' + B)
                S = pb.tile([P, F], f16, tag="S")
                y_pv = y[:].rearrange("p (h two) -> p two h", two=2)
                nc.scalar.activation(
                    S[:].rearrange("p (two h) -> p two h", two=2), y_pv, Sin,
                    bias=BIAS, scale=1.0)
                # half-angle planes: sin(y/2) via Sin(0.5*y' + B/2)
                Ha = pb.tile([P, F], f16, tag="Ha")
                nc.scalar.activation(
                    Ha[:].rearrange("p (two h) -> p two h", two=2), y_pv, Sin,
                    bias=BIAS2, scale=0.5)
                stash[j] = (S, Ha)

            def stage_c(k):
                S, Ha = stash.pop(k)
                # cc = 1 - 2*Ha^2 = [cos(x0) | cos(x1)]; squares split
                # across ACT (Square fn, same table set) and DVE (fp16 tt)
                sq = pb.tile([P, F], f16, tag="t")
                nc.scalar.activation(sq[:, 0:H], Ha[:, 0:H], Sq)
                nc.vector.tensor_tensor(sq[:, H:F], Ha[:, H:F], Ha[:, H:F],
                                        AluOpType.mult)
                cc = pb.tile([P, F], f16, tag="cc")
                nc.vector.tensor_scalar(
                    cc[:], sq[:], -2.0, None, AluOpType.mult)
                nc.vector.tensor_scalar(
                    cc[:], cc[:], 1.0, None, AluOpType.add)
                # q = ns*(sin(x0)*sin(x1)) + 0.5
                m = pb.tile([P, H], f16, tag="mg")
                nc.vector.tensor_tensor(m[:], S[:, 0:H], S[:, H:F],
                                        AluOpType.mult)
                nc.vector.tensor_scalar(
                    m[:], m[:], ns, 0.5, AluOpType.mult, AluOpType.add)
                g = pb.tile([P, H], f16, tag="mg")
                nc.vector.tensor_tensor(g[:], cc[:, 0:H], cc[:, H:F],
                                        AluOpType.mult)

                o = io.tile([P, F], f32, tag="x")
                ov = o[:].rearrange("p (h two) -> p h two", two=2)
                # out0 = hc*cos(x0) + q  (4x mult then 2x fp16->f32 add)
                r = pb.tile([P, H], f16, tag="r")
                nc.vector.tensor_scalar(
                    r[:], cc[:, 0:H], hc, None, AluOpType.mult)
                nc.vector.tensor_tensor(
                    ov[:, :, 0], r[:], m[:], AluOpType.add)
                # out1 = 0.5*g + 0.5  (fp16-in fp32-strided-out ts, 2x)
                nc.vector.tensor_scalar(
                    ov[:, :, 1], g[:], 0.5, 0.5, AluOpType.mult, AluOpType.add)
                nc.sync.dma_start(out=o_t[k], in_=o[:])

            for i in range(T + 2):
                if i < T:
                    stage_a(i)
                if 1 <= i < T + 1:
                    stage_b(i - 1)
                if i >= 2:
                    stage_c(i - 2)
    nc.compile()
    return nc


def _run(in_maps, trace=False, trace_cores=None):
    return bass_utils.run_bass_kernel_spmd(
        _CACHE["nc"],
        in_maps,
        core_ids=list(range(N_CORES)),
        trace=trace,
        trace_cores=trace_cores,
    )


def kernel(x, theta, _trace=False, _trace_cores=None):
    x = np.ascontiguousarray(np.asarray(x, dtype=np.float32))
    theta = np.asarray(theta, dtype=np.float32)
    assert x.shape == (B, 2), x.shape

    th = float(theta.reshape(-1)[0])
    hc = float(0.5 * np.cos(th))
    ns = float(-0.5 * np.sin(th))
    key = (hc, ns)
    if _CACHE.get("key") != key:
        _CACHE["nc"] = _build_nc(hc, ns)
        _CACHE["key"] = key

    shards = x.reshape(N_CORES, BC, 2)
    in_maps = [{"x": shards[c]} for c in range(N_CORES)]

    res = _run(in_maps, trace=_trace, trace_cores=_trace_cores)
    _CACHE["last_results"] = res
    out = np.concatenate([res.results[c]["out"] for c in range(N_CORES)], axis=0)
    return out


# revision 3
# speedup vs baseline: 1.0452x; 1.0307x over previous
"""Trainium2 Bass kernel for the 2-qubit quantum-circuit batch evaluation.

Math (per row, x = [x0, x1], scalar theta):
    out0 = 0.5 + 0.5*cos(theta)*cos(x0) - 0.5*sin(theta)*sin(x0)*sin(x1)
    out1 = 0.5 + 0.5*cos(x0)*cos(x1)

Design (all hw-measured; see transcript for the measurement history):
  * theta scalars baked as immediates (compile per theta; harness calls once).
  * range reduction with ZERO dedicated DVE rounding ops:
      - ACT Identity pass computes t16 = fp16(x/(2pi) + 1536); the fp16
        output-convert performs round-to-nearest-int (fp16 magic).
      - one fused cody_waite_cascade with MIXED dtypes (fp32 x, fp16 t16)
        gives y' = x - t16*2pi ~ y - 3072pi (exact-enough: t16*c1 is exact
        in fp32 since t16*402 < 2^24).
      - the -3072pi shift is cancelled FOR FREE by the Sin bias:
        sin(x) = Sin(y' + B), B = fp32(1536*(c1+c2+c3)); total err ~1e-3
        vs 2e-2 tolerance. Boundary k-flips are harmless (Sin accurate to
        ~1.25pi, hw-verified).
  * cos via half-angle square: cos(x) = 1 - 2*sin^2(y/2); the half-angle
    Sin pass is free (ACT scale=0.5, bias=B/2).
  * ACT Sin passes write fp16 PLANES (deinterleave x0/x1 via strided read,
    +7%) so all DVE combines run in 16-bit 2x/4x modes.
  * 3-stage software pipeline (engines execute in program order):
      a(i):   DMA in, ACT t16
      b(i-1): DVE cody, ACT Sin -> S planes, ACT Sin/2 -> Ha planes
      c(i-2): DVE squares+combines+out0, ACT out1, DMA out
    This keeps each engine's in-order stream free of same-tile round trips.

Per [128,2048] tile (8/core): DVE ~7.4us, ACT ~7.3us, DMA 2MB ~5.9us.
"""

import os
import numpy as np

import concourse.bass as bass
import concourse.mybir as mybir
from concourse.alu_op_type import AluOpType
from concourse.bacc import Bacc
from concourse.tile import TileContext
from concourse import bass_utils

N_CORES = 8
B = 8388608
BC = B // N_CORES            # rows per core
ELEMS = BC * 2               # flat f32 elements per core
P = 128
F = int(os.environ.get("TILE_F", "2048"))
H = F // 2
T = ELEMS // (P * F)         # tiles per core

INV_2PI = float(1.0 / (2 * np.pi))
M16 = 1536.0                 # fp16 round-to-int magic (1.5*2^10)
CW1 = float(np.float32(6.28125))
CW2 = float(np.float32(2 * np.pi - 6.28125))
CW3 = float(2 * np.pi - 6.28125 - float(np.float32(2 * np.pi - 6.28125)))
_CSUM = np.float64(CW1) + np.float64(CW2) + np.float64(CW3)
BIAS = float(np.float32(1536.0 * _CSUM))        # cancels the -3072pi shift
BIAS2 = float(np.float32(768.0 * _CSUM))

f32 = mybir.dt.float32
f16 = mybir.dt.float16
Sin = mybir.ActivationFunctionType.Sin
Sq = mybir.ActivationFunctionType.Square
Ident = mybir.ActivationFunctionType.Identity

_CACHE = {}


def _reg_consts(nc, vals):
    for v in vals:
        v = float(v)
        if (f32, v) in nc.const_aps.aps:
            continue
        t = nc.alloc_sbuf_tensor(f"const-float32-{v}", [128, 1], f32)
        nc.vector.memset(t.ap(), v)
        nc.const_aps.aps[(f32, v)] = t.ap()
    nc.all_engine_barrier()


def _build_nc(hc, ns):
    """hc = 0.5*cos(theta), ns = -0.5*sin(theta) baked as immediates."""
    nc = Bacc()
    _reg_consts(nc, [M16, BIAS, BIAS2, 0.5])
    x = nc.dram_tensor("x", [BC, 2], f32, kind="ExternalInput")
    out = nc.dram_tensor("out", [BC, 2], f32, kind="ExternalOutput")

    x_t = x[:].flatten().rearrange("(n p f) -> n p f", p=P, f=F)
    o_t = out[:].flatten().rearrange("(n p f) -> n p f", p=P, f=F)

    with TileContext(nc) as tc:
        with tc.tile_pool(name="io", bufs=8) as io, \
             tc.tile_pool(name="rk", bufs=6) as rk, \
             tc.tile_pool(name="pb", bufs=4) as pb:
            xts, t16s, ys, stash = {}, {}, {}, {}

            # dummy ACT op: trigger the Sin table load during the first DMA
            warm = rk.tile([P, 16], f16, tag="warm")
            nc.vector.memset(warm[:], 0.0)
            nc.scalar.activation(warm[:], warm[:], Sin)

            def stage_a(i):
                xt = io.tile([P, F], f32, tag="x")
                nc.sync.dma_start(out=xt[:], in_=x_t[i])
                # magic-round: t16 = fp16(x/(2pi) + 1536). First two tiles
                # on the (startup-idle) DVE so cody(0) isn't gated on the
                # ACT table load; steady state on ACT.
                t16 = pb.tile([P, F], f16, tag="t")
                if i < 2:
                    nc.vector.tensor_scalar(
                        t16[:], xt[:], INV_2PI, M16,
                        AluOpType.mult, AluOpType.add)
                else:
                    nc.scalar.activation(t16[:], xt[:], Ident,
                                         bias=M16, scale=INV_2PI)
                xts[i] = xt
                t16s[i] = t16

            def stage_b(j):
                xt = xts.pop(j)
                t16 = t16s.pop(j)
                # y' = x - t16*2pi ~ y - 3072pi  (mixed-dtype cody)
                y = rk.tile([P, F], f32, tag="y")
                nc.vector.cody_waite_cascade(y[:], xt[:], t16[:],
                                             CW1, CW2, CW3)
                # fp16 planes: S = [sin(x0) | sin(x1)] via Sin(y' + B)
                S = pb.tile([P, F], f16, tag="S")
                y_pv = y[:].rearrange("p (h two) -> p two h", two=2)
                nc.scalar.activation(
                    S[:].rearrange("p (two h) -> p two h", two=2), y_pv, Sin,
                    bias=BIAS, scale=1.0)
                # half-angle planes: sin(y/2) via Sin(0.5*y' + B/2)
                Ha = pb.tile([P, F], f16, tag="Ha")
                nc.scalar.activation(
                    Ha[:].rearrange("p (two h) -> p two h", two=2), y_pv, Sin,
                    bias=BIAS2, scale=0.5)
                stash[j] = (S, Ha)

            def stage_c(k):
                S, Ha = stash.pop(k)
                # cc = 1 - 2*Ha^2 = [cos(x0) | cos(x1)]; squares split
                # across ACT (Square fn, same table set) and DVE (fp16 tt)
                sq = pb.tile([P, F], f16, tag="t")
                nc.scalar.activation(sq[:, 0:H], Ha[:, 0:H], Sq)
                nc.vector.tensor_tensor(sq[:, H:F], Ha[:, H:F], Ha[:, H:F],
                                        AluOpType.mult)
                cc = pb.tile([P, F], f16, tag="cc")
                nc.vector.tensor_scalar(
                    cc[:], sq[:], -2.0, 1.0, AluOpType.mult, AluOpType.add)
                # q = ns*(sin(x0)*sin(x1)) + 0.5
                m = pb.tile([P, H], f16, tag="mg")
                nc.vector.tensor_tensor(m[:], S[:, 0:H], S[:, H:F],
                                        AluOpType.mult)
                nc.vector.tensor_scalar(
                    m[:], m[:], ns, 0.5, AluOpType.mult, AluOpType.add)
                g = pb.tile([P, H], f16, tag="mg")
                nc.vector.tensor_tensor(g[:], cc[:, 0:H], cc[:, H:F],
                                        AluOpType.mult)

                o = io.tile([P, F], f32, tag="x")
                ov = o[:].rearrange("p (h two) -> p h two", two=2)
                # out0 = hc*cos(x0) + q  (4x mult then 2x fp16->f32 add)
                r = pb.tile([P, H], f16, tag="r")
                nc.vector.tensor_scalar(
                    r[:], cc[:, 0:H], hc, None, AluOpType.mult)
                nc.vector.tensor_tensor(
                    ov[:, :, 0], r[:], m[:], AluOpType.add)
                # out1 = 0.5*g + 0.5  (fp16-in fp32-strided-out ts, 2x)
                nc.vector.tensor_scalar(
                    ov[:, :, 1], g[:], 0.5, 0.5, AluOpType.mult, AluOpType.add)
                nc.sync.dma_start(out=o_t[k], in_=o[:])

            for i in range(T + 2):
                if i < T:
                    stage_a(i)
                if 1 <= i < T + 1:
                    stage_b(i - 1)
                if i >= 2:
                    stage_c(i - 2)
    nc.compile()
    return nc


def _run(in_maps, trace=False, trace_cores=None):
    return bass_utils.run_bass_kernel_spmd(
        _CACHE["nc"],
        in_maps,
        core_ids=list(range(N_CORES)),
        trace=trace,
        trace_cores=trace_cores,
    )


def kernel(x, theta, _trace=False, _trace_cores=None):
    x = np.ascontiguousarray(np.asarray(x, dtype=np.float32))
    theta = np.asarray(theta, dtype=np.float32)
    assert x.shape == (B, 2), x.shape

    th = float(theta.reshape(-1)[0])
    hc = float(0.5 * np.cos(th))
    ns = float(-0.5 * np.sin(th))
    key = (hc, ns)
    if _CACHE.get("key") != key:
        _CACHE["nc"] = _build_nc(hc, ns)
        _CACHE["key"] = key

    shards = x.reshape(N_CORES, BC, 2)
    in_maps = [{"x": shards[c]} for c in range(N_CORES)]

    res = _run(in_maps, trace=_trace, trace_cores=_trace_cores)
    _CACHE["last_results"] = res
    out = np.concatenate([res.results[c]["out"] for c in range(N_CORES)], axis=0)
    return out
